# revision 11
# baseline (speedup 1.0000x reference)
"""Beam-search top-k (k=16) Trainium2 Bass kernel.

Computation (per batch row b):
  keep[b,m]  = all(mask[b,m,:] != 0)
  val[b,m,v] = keep ? lprobs[b,m,v] + s[b,m] : s[b,m]      (s = scores[:,:,step-1])
  top-16 of val flattened over (m, v), ties -> lowest flat index,
  returning (values, vocab_idx, beam_idx).

Strategy (8 NeuronCores, batch-sharded, 8 rows/core):
  - per-core shard lprobs (8, 8, 50257), each beam padded to VB=50688 with
    -1e30 so every DMA granule is 256B-aligned and beam stride is uniform
  - SBUF layout: 128 partitions = (h half, r row, m beam) x 25344 f32
  - load runs on 6 parallel ~2.1MB DMA streams: SP + Act HWDGE dma_starts
    plus 8 dma_gather calls spread over SWDGE queues 0..3 (each DMA queue
    sustains only ~56 GB/s; six queues ~ 340 GB/s)
  - stage 1: grouped max (g=32) on DVE per 2112-col block -> G (128, 792)
  - stage 2: G' = G*keep + s; per-partition top-16 groups (max8/max_index/
    match_replace x2); candidates bounced through DRAM into per-row (8, 256)
    tables; top-16 winning groups per row
  - stage 3: indirect-DMA gather of the winning 32-elem groups straight
    from HBM (partition q = (row, slot)); re-apply keep/score
  - stage 4: per-group top-16 -> 256 candidates/row -> final top-16/row
  First-match semantics of max_index/match_replace reproduce lax.top_k's
  lowest-index tie-breaking (masked beams are constant rows -> huge ties).
"""

import os
import sys

import numpy as np

sys.path.insert(0, "/opt/trn_rl_repo")

VOCAB = 50257
BEAM = 8
BSZ = 64
NGRAM = 4
K = 16
NCORES = 8
RPC = BSZ // NCORES          # rows (batch entries) per core
VB = 50688                   # padded per-beam length (multiple of 2112)
HALFB = VB // 2              # 25344 elements per partition
ROWPAD = BEAM * VB           # 405504 padded flat row length
G = 32                       # group size (elements)
NGH = HALFB // G             # 792 groups per partition
BLK = 2112                   # load-block columns (= 66 groups)
NBLK = HALFB // BLK          # 12 blocks per partition
NEG = -1.0e30

# SWDGE gather calls: (column_base, width==elem_size, queue).  Emission
# order fixes the DMASW lane (idx % 8); the 3 indirect DMAs that follow
# land on lanes 0..2, so those lanes' gathers must share queue 0.
GCALLS = [
    (8448, 1408, 0), (9856, 1408, 0), (11264, 1408, 0),
    (12672, 2112, 1), (14784, 2112, 2), (16896, 4224, 3),
    (21120, 2112, 1), (23232, 2112, 2),
]

_CACHE = {}


def _consts():
    """Shape-derived constant inputs.  partition p = h*64 + r*8 + m."""
    p = np.arange(128)
    h = p // 64
    m = p % 8
    pc = np.zeros((128, 2), np.float32)
    pc[:, 0] = m * VB + h * HALFB       # element offset of the partition's block
    pc[:, 1] = m                         # beam id
    q = np.arange(128)                   # gathered-partition q = r*16 + slot
    qc = np.zeros((128, 1), np.float32)
    qc[:, 0] = (q // 16) * ROWPAD        # row base for the main gather

    # dma_gather row tables: one call per column-piece, idx wrapped
    # (16, 8) then replicated down the 128 partitions.  The row id of
    # partition p for a call with elem E based at column a is
    # ((p%64)*VB + (p//64)*HALFB) // E  (in_ap view starts at offset a).
    gidx = np.zeros((128, 8 * 8), np.int16)
    for j, (a, E, _q) in enumerate(GCALLS):
        rid = ((np.arange(128) % 64) * VB + (np.arange(128) // 64) * HALFB) // E
        blk = rid.reshape(8, 16).T                  # [q%16, q//16]
        gidx[:, j * 8:(j + 1) * 8] = np.tile(blk, (8, 1))
    return pc, qc, gidx


def _build(debug=False):
    """Build + compile the Bass program (cached per process)."""
    key = ("nc", debug)
    if key in _CACHE:
        return _CACHE[key]

    import concourse.bacc as bacc
    import concourse.tile as tile
    from concourse import mybir
    from concourse.ap import AP
    import concourse.bass as bass

    f32 = mybir.dt.float32
    i16 = mybir.dt.int16
    i32 = mybir.dt.int32
    u32 = mybir.dt.uint32
    X = mybir.AxisListType.X
    op = mybir.AluOpType

    nc = bacc.Bacc("TRN2", target_bir_lowering=False, debug=False,
                   num_devices=NCORES, num_swdge_queues=4)

    lp = nc.dram_tensor("lp", [RPC, ROWPAD], f32, kind="ExternalInput")
    sv = nc.dram_tensor("sv", [128, 1], f32, kind="ExternalInput")
    mk = nc.dram_tensor("mk", [128, NGRAM], i32, kind="ExternalInput")
    sq = nc.dram_tensor("sq", [128, BEAM], f32, kind="ExternalInput")
    pc = nc.dram_tensor("pc", [128, 2], f32, kind="ExternalInput")
    qc = nc.dram_tensor("qc", [128, 1], f32, kind="ExternalInput")
    gi = nc.dram_tensor("gi", [128, 64], i16, kind="ExternalInput")
    ov = nc.dram_tensor("ov", [RPC, K], f32, kind="ExternalOutput")
    oi = nc.dram_tensor("oi", [RPC, K], i32, kind="ExternalOutput")
    ob = nc.dram_tensor("ob", [RPC, K], i32, kind="ExternalOutput")

    cdump = nc.dram_tensor("cdump", [128, 16], f32)    # cand values
    cpdump = nc.dram_tensor("cpdump", [128, 32], f32)  # (off,beam) pairs
    atab = nc.dram_tensor("atab", [RPC, 256, 2], f32)  # per-row (off,beam)
    kdump = nc.dram_tensor("kdump", [128, 1], f32)
    aod = nc.dram_tensor("aod", [RPC, 16], i32)        # attr-offset staging
    fod = nc.dram_tensor("fod", [RPC, 16], i32)        # final-offset staging
    fdump = nc.dram_tensor("fdump", [128, 16], f32)    # final cand values
    fpdump = nc.dram_tensor("fpdump", [128, 32], f32)  # (beam,vocab) pairs

    with tile.TileContext(nc) as tc:
        from contextlib import ExitStack

        ctx = ExitStack()
        sb = ctx.enter_context(tc.tile_pool(name="persist", bufs=1))

        v = nc.vector
        sc = nc.scalar
        gp_ = nc.gpsimd

        # ---- small input loads (off the big-load queues' critical path) --
        sv_t = sb.tile([128, 1], f32)
        sc.dma_start(sv_t[:], sv[:])
        mk_t = sb.tile([128, NGRAM], i32)
        sc.dma_start(mk_t[:], mk[:])
        pc_t = sb.tile([128, 2], f32)
        sc.dma_start(pc_t[:], pc[:])
        qc_t = sb.tile([128, 1], f32)
        sc.dma_start(qc_t[:], qc[:])
        gi_t = sb.tile([128, 64], i16)
        nc.sync.dma_start(gi_t[:], gi[:])
        srow = sb.tile([128, 8], f32)
        sc.dma_start(srow[:], sq[:])

        # keep = all(mask != 0), as 0.0/1.0; bounce for the q-layout table
        mkf = sb.tile([128, NGRAM], f32)
        v.tensor_copy(mkf[:], mk_t[:])
        keep = sb.tile([128, 1], f32)
        v.tensor_reduce(keep[:], mkf[:], axis=X, op=op.min)
        v.tensor_scalar(keep[:], keep[:], 0.5, None, op0=op.is_ge)
        nc.sync.dma_start(kdump[:], keep[:])
        krow = sb.tile([128, 8], f32)
        sc.dma_start(krow[:], AP(kdump, 0, [[8, 8], [0, 16], [1, 8]]))

        # ---- stage 1: 6-stream load + per-block grouped max --------------
        xt = sb.tile([128, HALFB], f32)
        gg = sb.tile([128, NGH], f32)

        def reduce_blocks(b0, nb):
            v.tensor_reduce(
                gg[:, b0 * (BLK // G):(b0 + nb) * (BLK // G)],
                xt[:, b0 * BLK:(b0 + nb) * BLK].rearrange(
                    "p (n g) -> p n g", g=G),
                axis=X,
                op=op.max,
            )

        def reduce_cols(a, w):
            v.tensor_reduce(
                gg[:, a // G:(a + w) // G],
                xt[:, a:a + w].rearrange("p (n g) -> p n g", g=G),
                axis=X,
                op=op.max,
            )

        # HWDGE pieces on SP and Act (4224 cols each)
        nc.sync.dma_start(
            xt[:, 0:4224],
            AP(lp, 0, [[HALFB, 2], [VB, 64], [1, 4224]]),
        )
        reduce_cols(0, 4224)
        sc.dma_start(
            xt[:, 4224:8448],
            AP(lp, 4224, [[HALFB, 2], [VB, 64], [1, 4224]]),
        )
        reduce_cols(4224, 4224)
        # SWDGE gather pieces; emission order fixes the DMASW lanes
        total = RPC * ROWPAD
        for j, (a, E, qn) in enumerate(GCALLS):
            gp_.dma_gather(
                out_ap=xt[:, a:a + E].rearrange("p (i e) -> p i e", i=1),
                in_ap=AP(lp, a, [[E, (total - a) // E], [1, E]]),
                idxs_ap=gi_t[:, j * 8:(j + 1) * 8],
                num_idxs=128,
                num_idxs_reg=128,
                elem_size=E,
                queue_num=qn,
            )
            reduce_cols(a, E)

        # ---- stage 2a: G' and per-partition top-16 groups ----------------
        gpv = sb.tile([128, NGH], f32)
        v.tensor_scalar(gpv[:], gg[:], keep[:, 0:1], sv_t[:, 0:1],
                        op0=op.mult, op1=op.add)
        cand = sb.tile([128, 16], f32)
        candp = sb.tile([128, 32], f32)    # interleaved (off, beam) pairs
        ci = sb.tile([128, 16], u32)
        gz = sb.tile([128, NGH], f32)
        v.max(cand[:, 0:8], gpv[:])
        v.max_index(ci[:, 0:8], cand[:, 0:8], gpv[:])
        v.match_replace(gz[:], in_to_replace=cand[:, 0:8], in_values=gpv[:],
                        imm_value=NEG)
        v.max(cand[:, 8:16], gz[:])
        v.max_index(ci[:, 8:16], cand[:, 8:16], gz[:])

        cif = sb.tile([128, 16], f32)
        v.tensor_copy(cif[:], ci[:])
        cpv = candp[:].rearrange("p (k c) -> p c k", c=2)
        # off = local*32 + (m*VB + h*HALFB)
        v.tensor_scalar(cpv[:, 0:1, :].squeeze(1), cif[:], float(G),
                        pc_t[:, 0:1], op0=op.mult, op1=op.add)
        v.tensor_copy(cpv[:, 1:2, :].squeeze(1),
                      pc_t[:, 1:2].to_broadcast([128, 16]))

        # ---- bounce candidates to per-row layout -------------------------
        nc.sync.dma_start(cdump[:], cand[:])
        sc.dma_start(cpdump[:], candp[:])
        cv = sb.tile([8, 256], f32)
        for h in range(2):
            nc.sync.dma_start(cv[:, h * 128:(h + 1) * 128],
                              AP(cdump, h * 1024, [[128, 8], [16, 8], [1, 16]]))
            sc.dma_start(AP(atab, h * 256, [[512, 8], [32, 8], [1, 32]]),
                         AP(cpdump, h * 2048, [[256, 8], [32, 8], [1, 32]]))

        # ---- stage 2b: top-16 winning groups per row ---------------------
        wv = sb.tile([8, 16], f32)
        wpos = sb.tile([8, 16], u32)
        cz = sb.tile([8, 256], f32)
        v.max(wv[:, 0:8], cv[:])
        v.max_index(wpos[:, 0:8], wv[:, 0:8], cv[:])
        v.match_replace(cz[:], in_to_replace=wv[:, 0:8], in_values=cv[:],
                        imm_value=NEG)
        v.max(wv[:, 8:16], cz[:])
        v.max_index(wpos[:, 8:16], wv[:, 8:16], cz[:])

        io16 = sb.tile([8, 16], i32)
        gp_.iota(io16[:], pattern=[[0, 16]], base=0, channel_multiplier=512)
        io16f = sb.tile([8, 16], f32)
        v.tensor_copy(io16f[:], io16[:])
        wposf = sb.tile([8, 16], f32)
        v.tensor_copy(wposf[:], wpos[:])
        aofs_f = sb.tile([8, 16], f32)
        v.tensor_scalar(aofs_f[:], wposf[:], 2.0, None, op0=op.mult)
        v.tensor_tensor(aofs_f[:], aofs_f[:], io16f[:], op=op.add)
        aofs = sb.tile([8, 16], i32)
        v.tensor_copy(aofs[:], aofs_f[:])
        nc.sync.dma_start(aod[:], aofs[:])
        aofsq = sb.tile([128, 1], i32)
        nc.sync.dma_start(aofsq[:], AP(aod, 0, [[1, 128], [1, 1]]))

        # gather (off, beam) of each winning group -> partition q=(r,slot)
        attr = sb.tile([128, 2], f32)
        gp_.indirect_dma_start(
            out=attr[:],
            out_offset=None,
            in_=AP(atab, 0, [[1, RPC * 256 * 2], [1, 1]]),
            in_offset=bass.IndirectOffsetOnAxis(ap=aofsq[:, 0:1], axis=0),
        )

        # per-winning-group s and keep via beam-id one-hot
        iom = sb.tile([128, 8], i32)
        gp_.iota(iom[:], pattern=[[1, 8]], base=0, channel_multiplier=0)
        iomf = sb.tile([128, 8], f32)
        v.tensor_copy(iomf[:], iom[:])
        eq = sb.tile([128, 8], f32)
        v.tensor_tensor(eq[:], attr[:, 1:2].to_broadcast([128, 8]), iomf[:],
                        op=op.is_equal)
        tmp8 = sb.tile([128, 8], f32)
        v.tensor_tensor(tmp8[:], eq[:], srow[:], op=op.mult)
        ws = sb.tile([128, 1], f32)
        v.tensor_reduce(ws[:], tmp8[:], axis=X, op=op.add)
        v.tensor_tensor(tmp8[:], eq[:], krow[:], op=op.mult)
        wk = sb.tile([128, 1], f32)
        v.tensor_reduce(wk[:], tmp8[:], axis=X, op=op.add)

        # ---- stage 3: gather winning groups from HBM ---------------------
        gofs_f = sb.tile([128, 1], f32)
        v.tensor_tensor(gofs_f[:], attr[:, 0:1], qc_t[:], op=op.add)
        gofs = sb.tile([128, 1], i32)
        v.tensor_copy(gofs[:], gofs_f[:])
        grp = sb.tile([128, G], f32)
        gp_.indirect_dma_start(
            out=grp[:],
            out_offset=None,
            in_=AP(lp, 0, [[1, RPC * ROWPAD], [1, 1]]),
            in_offset=bass.IndirectOffsetOnAxis(ap=gofs[:, 0:1], axis=0),
        )
        base = sb.tile([128, G], f32)
        sc.activation(base[:], grp[:], mybir.ActivationFunctionType.Identity,
                      bias=ws[:, 0:1], scale=wk[:, 0:1])

        # ---- stage 4a: per-group top-16 ----------------------------------
        fval = sb.tile([128, 16], f32)
        finp = sb.tile([128, 32], f32)     # interleaved (beam, vocab) pairs
        gl = sb.tile([128, 16], u32)
        bz = sb.tile([128, G], f32)
        v.max(fval[:, 0:8], base[:])
        v.max_index(gl[:, 0:8], fval[:, 0:8], base[:])
        v.match_replace(bz[:], in_to_replace=fval[:, 0:8], in_values=base[:],
                        imm_value=NEG)
        v.max(fval[:, 8:16], bz[:])
        v.max_index(gl[:, 8:16], fval[:, 8:16], bz[:])

        glf = sb.tile([128, 16], f32)
        v.tensor_copy(glf[:], gl[:])
        t1 = sb.tile([128, 1], f32)
        v.tensor_scalar(t1[:], attr[:, 1:2], float(VB), None, op0=op.mult)
        vb = sb.tile([128, 1], f32)
        v.tensor_tensor(vb[:], attr[:, 0:1], t1[:], op=op.subtract)
        fpv = finp[:].rearrange("p (k c) -> p c k", c=2)
        v.tensor_copy(fpv[:, 0:1, :].squeeze(1),
                      attr[:, 1:2].to_broadcast([128, 16]))
        v.tensor_scalar(fpv[:, 1:2, :].squeeze(1), glf[:], vb[:, 0:1], None,
                        op0=op.add)

        # ---- bounce final candidates (identity layout) -------------------
        nc.sync.dma_start(fdump[:], fval[:])
        sc.dma_start(fpdump[:], finp[:])
        fv = sb.tile([8, 256], f32)
        nc.sync.dma_start(fv[:], AP(fdump, 0, [[256, 8], [1, 256]]))

        # ---- stage 4c: final top-16 per row ------------------------------
        FV = sb.tile([8, 16], f32)
        fpos = sb.tile([8, 16], u32)
        fz = sb.tile([8, 256], f32)
        v.max(FV[:, 0:8], fv[:])
        v.max_index(fpos[:, 0:8], FV[:, 0:8], fv[:])
        v.match_replace(fz[:], in_to_replace=FV[:, 0:8], in_values=fv[:],
                        imm_value=NEG)
        v.max(FV[:, 8:16], fz[:])
        v.max_index(fpos[:, 8:16], FV[:, 8:16], fz[:])

        fposf = sb.tile([8, 16], f32)
        v.tensor_copy(fposf[:], fpos[:])
        fofs_f = sb.tile([8, 16], f32)
        v.tensor_scalar(fofs_f[:], fposf[:], 2.0, None, op0=op.mult)
        v.tensor_tensor(fofs_f[:], fofs_f[:], io16f[:], op=op.add)
        fofs = sb.tile([8, 16], i32)
        v.tensor_copy(fofs[:], fofs_f[:])
        nc.sync.dma_start(fod[:], fofs[:])
        fofsq = sb.tile([128, 1], i32)
        nc.sync.dma_start(fofsq[:], AP(fod, 0, [[1, 128], [1, 1]]))
        fattr = sb.tile([128, 2], f32)
        gp_.indirect_dma_start(
            out=fattr[:],
            out_offset=None,
            in_=AP(fpdump, 0, [[1, 128 * 32], [1, 1]]),
            in_offset=bass.IndirectOffsetOnAxis(ap=fofsq[:, 0:1], axis=0),
        )
        fbi = sb.tile([128, 1], i32)
        v.tensor_copy(fbi[:], fattr[:, 0:1])
        fvi = sb.tile([128, 1], i32)
        v.tensor_copy(fvi[:], fattr[:, 1:2])

        if debug:
            for nm, t in [("d_gp", gpv), ("d_cand", cand), ("d_candp", candp),
                          ("d_cv", cv), ("d_wv", wv), ("d_attr", attr),
                          ("d_krow", krow), ("d_ws", ws), ("d_wk", wk),
                          ("d_grp", grp), ("d_base", base), ("d_fval", fval),
                          ("d_finp", finp), ("d_fv", fv)]:
                dt_ = nc.dram_tensor(nm, list(t[:].shape), f32,
                                     kind="ExternalOutput")
                sc.dma_start(dt_[:], t[:])
            for nm, t in [("d_aofsq", aofsq), ("d_fofsq", fofsq)]:
                dt_ = nc.dram_tensor(nm, list(t[:].shape), i32,
                                     kind="ExternalOutput")
                sc.dma_start(dt_[:], t[:])

        # ---- outputs -----------------------------------------------------
        nc.sync.dma_start(ov[:], FV[:])
        sc.dma_start(AP(ob, 0, [[1, 128], [1, 1]]), fbi[:])
        nc.sync.dma_start(AP(oi, 0, [[1, 128], [1, 1]]), fvi[:])

        ctx.close()

    nc.compile()
    _CACHE[key] = nc
    return nc


def _prep_inputs(lprobs, scores, mask, step):
    """Host-side shard + marshal. Returns in_maps for the 8 cores."""
    lprobs = np.asarray(lprobs, np.float32)
    scores = np.asarray(scores, np.float32)
    mask = np.ascontiguousarray(np.asarray(mask, np.int32))
    step = int(step)

    if step == 0:
        s2d = np.zeros((BSZ, BEAM), np.float32)
        s2d[:, 1:] = NEG
        mask = mask.copy()
        mask[:, 1:, :] = 0           # force beams 1.. masked with s=NEG
    else:
        s2d = np.ascontiguousarray(scores[:, :, step - 1])

    flat = np.full((BSZ, BEAM, VB), NEG, np.float32)
    flat[:, :, :VOCAB] = lprobs
    flat = flat.reshape(BSZ, ROWPAD)

    pcc, qcc, gidx = _consts()
    p = np.arange(128)
    ph, pr, pm = p // 64, (p // 8) % 8, p % 8      # p = h*64 + r*8 + m
    qr = np.arange(128) // 16                       # q = r*16 + slot
    in_maps = []
    for c in range(NCORES):
        rs = slice(c * RPC, (c + 1) * RPC)
        s_sh = s2d[rs]
        in_maps.append({
            "lp": np.ascontiguousarray(flat[rs]),
            "sv": np.ascontiguousarray(s_sh[pr, pm][:, None]),
            "mk": np.ascontiguousarray(mask[rs][pr, pm]),
            "sq": np.ascontiguousarray(s_sh[qr]),
            "pc": pcc,
            "qc": qcc,
            "gi": gidx,
        })
    return in_maps


def kernel(lprobs, scores, mask, step):
    from concourse.bass_utils import run_bass_kernel_spmd

    nc = _build()
    in_maps = _prep_inputs(lprobs, scores, mask, step)
    res = run_bass_kernel_spmd(nc, in_maps, list(range(NCORES))).results

    vals = np.concatenate([r["ov"] for r in res], axis=0)
    vocab = np.concatenate([r["oi"] for r in res], axis=0)
    beams = np.concatenate([r["ob"] for r in res], axis=0)
    return vals, vocab.astype(np.int32), beams.astype(np.int32)


# revision 14
# speedup vs baseline: 1.4544x; 1.4544x over previous
"""Beam-search top-k (k=16) Trainium2 Bass kernel.

Computation (per batch row b):
  keep[b,m]  = all(mask[b,m,:] != 0)
  val[b,m,v] = keep ? lprobs[b,m,v] + s[b,m] : s[b,m]      (s = scores[:,:,step-1])
  top-16 of val flattened over (m, v), ties -> lowest flat index,
  returning (values, vocab_idx, beam_idx).

Strategy (8 NeuronCores, batch-sharded, 8 rows/core):
  - per-core shard lprobs (8, 8, 50257), each beam padded to VB=50688 with
    -1e30 so every DMA granule is 256B-aligned and beam stride is uniform
  - SBUF layout: 128 partitions = (h half, r row, m beam) x 25344 f32
  - load runs on 6 parallel ~2.1MB DMA streams: SP + Act HWDGE dma_starts
    plus 8 dma_gather calls spread over SWDGE queues 0..3 (each DMA queue
    sustains only ~56 GB/s; six queues ~ 340 GB/s)
  - stage 1: grouped max (g=32) on DVE per 2112-col block -> G (128, 792)
  - stage 2: G' = G*keep + s; per-partition top-16 groups (max8/max_index/
    match_replace x2); candidates bounced through DRAM into per-row (8, 256)
    tables; top-16 winning groups per row
  - stage 3: indirect-DMA gather of the winning 32-elem groups straight
    from HBM (partition q = (row, slot)); re-apply keep/score
  - stage 4: per-group top-16 -> 256 candidates/row -> final top-16/row
  First-match semantics of max_index/match_replace reproduce lax.top_k's
  lowest-index tie-breaking (masked beams are constant rows -> huge ties).
"""

import os
import sys

import numpy as np

sys.path.insert(0, "/opt/trn_rl_repo")

VOCAB = 50257
BEAM = 8
BSZ = 64
NGRAM = 4
K = 16
NCORES = 8
RPC = BSZ // NCORES          # rows (batch entries) per core
VB = 50688                   # padded per-beam length (multiple of 2112)
HALFB = VB // 2              # 25344 elements per partition
ROWPAD = BEAM * VB           # 405504 padded flat row length
G = 32                       # group size (elements)
NGH = HALFB // G             # 792 groups per partition
BLK = 2112                   # load-block columns (= 66 groups)
NBLK = HALFB // BLK          # 12 blocks per partition
NEG = -1.0e30

# SWDGE gather calls: (column_base, width==elem_size, queue).  The whole
# load runs on SWDGE (HWDGE descriptor generation is ~0.5us/descriptor and
# caps a queue near 25 GB/s; SWDGE sprays 16 DMA engines per queue).
# Emission order fixes the DMASW lane (idx % 8); the 3 indirect DMAs that
# follow land on lanes 4..6, so those lanes' gathers must use queue 0.
# 12 gathers -> lanes j%8; the 3 indirect DMAs then land on lanes 4,5,6,
# so j=4,5,6 must be queue 0; j8..j11 must match j0..j3's queues.
GCALLS = [
    (0, 2112, 1), (2112, 2112, 2), (4224, 4224, 3), (8448, 2112, 1),
    (10560, 1408, 0), (11968, 1408, 0), (13376, 1408, 0), (14784, 2816, 2),
    (17600, 1408, 1), (19008, 2112, 2), (21120, 2816, 3), (23936, 1408, 1),
]

_CACHE = {}


def _consts():
    """Shape-derived constant inputs.  partition p = h*64 + r*8 + m."""
    p = np.arange(128)
    h = p // 64
    m = p % 8
    pc = np.zeros((128, 2), np.float32)
    pc[:, 0] = m * VB + h * HALFB       # element offset of the partition's block
    pc[:, 1] = m                         # beam id
    q = np.arange(128)                   # gathered-partition q = r*16 + slot
    qc = np.zeros((128, 1), np.float32)
    qc[:, 0] = (q // 16) * ROWPAD        # row base for the main gather

    # dma_gather row tables: one call per column-piece, idx wrapped
    # (16, 8) then replicated down the 128 partitions.  The row id of
    # partition p for a call with elem E based at column a is
    # ((p%64)*VB + (p//64)*HALFB) // E  (in_ap view starts at offset a).
    gidx = np.zeros((128, 12 * 8), np.int16)
    for j, (a, E, _q) in enumerate(GCALLS):
        rid = ((np.arange(128) % 64) * VB + (np.arange(128) // 64) * HALFB) // E
        blk = rid.reshape(8, 16).T                  # [q%16, q//16]
        gidx[:, j * 8:(j + 1) * 8] = np.tile(blk, (8, 1))
    return pc, qc, gidx


def _build(debug=False):
    """Build + compile the Bass program (cached per process)."""
    key = ("nc", debug)
    if key in _CACHE:
        return _CACHE[key]

    import concourse.bacc as bacc
    import concourse.tile as tile
    from concourse import mybir
    from concourse.ap import AP
    import concourse.bass as bass

    f32 = mybir.dt.float32
    i16 = mybir.dt.int16
    i32 = mybir.dt.int32
    u32 = mybir.dt.uint32
    X = mybir.AxisListType.X
    op = mybir.AluOpType

    nc = bacc.Bacc("TRN2", target_bir_lowering=False, debug=False,
                   num_devices=NCORES, num_swdge_queues=4)

    lp = nc.dram_tensor("lp", [RPC, ROWPAD], f32, kind="ExternalInput")
    sv = nc.dram_tensor("sv", [128, 1], f32, kind="ExternalInput")
    mk = nc.dram_tensor("mk", [128, NGRAM], i32, kind="ExternalInput")
    sq = nc.dram_tensor("sq", [128, BEAM], f32, kind="ExternalInput")
    pc = nc.dram_tensor("pc", [128, 2], f32, kind="ExternalInput")
    qc = nc.dram_tensor("qc", [128, 1], f32, kind="ExternalInput")
    gi = nc.dram_tensor("gi", [128, 96], i16, kind="ExternalInput")
    ov = nc.dram_tensor("ov", [RPC, K], f32, kind="ExternalOutput")
    oi = nc.dram_tensor("oi", [RPC, K], i32, kind="ExternalOutput")
    ob = nc.dram_tensor("ob", [RPC, K], i32, kind="ExternalOutput")

    cdump = nc.dram_tensor("cdump", [128, 16], f32)    # cand values
    cpdump = nc.dram_tensor("cpdump", [128, 32], f32)  # (off,beam) pairs
    atab = nc.dram_tensor("atab", [RPC, 256, 2], f32)  # per-row (off,beam)
    kdump = nc.dram_tensor("kdump", [128, 1], f32)
    aod = nc.dram_tensor("aod", [RPC, 16], i32)        # attr-offset staging
    fod = nc.dram_tensor("fod", [RPC, 16], i32)        # final-offset staging
    fdump = nc.dram_tensor("fdump", [128, 16], f32)    # final cand values
    fpdump = nc.dram_tensor("fpdump", [128, 32], f32)  # (beam,vocab) pairs

    with tile.TileContext(nc) as tc:
        from contextlib import ExitStack

        ctx = ExitStack()
        sb = ctx.enter_context(tc.tile_pool(name="persist", bufs=1))

        v = nc.vector
        sc = nc.scalar
        gp_ = nc.gpsimd

        # ---- small input loads (off the big-load queues' critical path) --
        sv_t = sb.tile([128, 1], f32)
        sc.dma_start(sv_t[:], sv[:])
        mk_t = sb.tile([128, NGRAM], i32)
        sc.dma_start(mk_t[:], mk[:])
        pc_t = sb.tile([128, 2], f32)
        sc.dma_start(pc_t[:], pc[:])
        qc_t = sb.tile([128, 1], f32)
        sc.dma_start(qc_t[:], qc[:])
        gi_t = sb.tile([128, 96], i16)
        nc.sync.dma_start(gi_t[:], gi[:])
        srow = sb.tile([128, 8], f32)
        sc.dma_start(srow[:], sq[:])

        # keep = all(mask != 0), as 0.0/1.0; bounce for the q-layout table
        mkf = sb.tile([128, NGRAM], f32)
        v.tensor_copy(mkf[:], mk_t[:])
        keep = sb.tile([128, 1], f32)
        v.tensor_reduce(keep[:], mkf[:], axis=X, op=op.min)
        v.tensor_scalar(keep[:], keep[:], 0.5, None, op0=op.is_ge)
        nc.sync.dma_start(kdump[:], keep[:])
        krow = sb.tile([128, 8], f32)
        sc.dma_start(krow[:], AP(kdump, 0, [[8, 8], [0, 16], [1, 8]]))

        # ---- stage 1: 6-stream load + per-block grouped max --------------
        xt = sb.tile([128, HALFB], f32)
        gg = sb.tile([128, NGH], f32)

        def reduce_blocks(b0, nb):
            v.tensor_reduce(
                gg[:, b0 * (BLK // G):(b0 + nb) * (BLK // G)],
                xt[:, b0 * BLK:(b0 + nb) * BLK].rearrange(
                    "p (n g) -> p n g", g=G),
                axis=X,
                op=op.max,
            )

        def reduce_cols(a, w):
            v.tensor_reduce(
                gg[:, a // G:(a + w) // G],
                xt[:, a:a + w].rearrange("p (n g) -> p n g", g=G),
                axis=X,
                op=op.max,
            )

        # SWDGE gather pieces; emission order fixes the DMASW lanes
        total = RPC * ROWPAD
        for j, (a, E, qn) in enumerate(GCALLS):
            gp_.dma_gather(
                out_ap=xt[:, a:a + E].rearrange("p (i e) -> p i e", i=1),
                in_ap=AP(lp, a, [[E, (total - a) // E], [1, E]]),
                idxs_ap=gi_t[:, j * 8:(j + 1) * 8],
                num_idxs=128,
                num_idxs_reg=128,
                elem_size=E,
                queue_num=qn,
            )
            reduce_cols(a, E)

        # ---- stage 2a: G' and per-partition top-16 groups ----------------
        gpv = sb.tile([128, NGH], f32)
        v.tensor_scalar(gpv[:], gg[:], keep[:, 0:1], sv_t[:, 0:1],
                        op0=op.mult, op1=op.add)
        cand = sb.tile([128, 16], f32)
        candp = sb.tile([128, 32], f32)    # interleaved (off, beam) pairs
        ci = sb.tile([128, 16], u32)
        gz = sb.tile([128, NGH], f32)
        v.max(cand[:, 0:8], gpv[:])
        v.max_index(ci[:, 0:8], cand[:, 0:8], gpv[:])
        v.match_replace(gz[:], in_to_replace=cand[:, 0:8], in_values=gpv[:],
                        imm_value=NEG)
        v.max(cand[:, 8:16], gz[:])
        v.max_index(ci[:, 8:16], cand[:, 8:16], gz[:])

        cif = sb.tile([128, 16], f32)
        v.tensor_copy(cif[:], ci[:])
        cpv = candp[:].rearrange("p (k c) -> p c k", c=2)
        # off = local*32 + (m*VB + h*HALFB)
        v.tensor_scalar(cpv[:, 0:1, :].squeeze(1), cif[:], float(G),
                        pc_t[:, 0:1], op0=op.mult, op1=op.add)
        v.tensor_copy(cpv[:, 1:2, :].squeeze(1),
                      pc_t[:, 1:2].to_broadcast([128, 16]))

        # ---- bounce candidates to per-row layout -------------------------
        nc.sync.dma_start(cdump[:], cand[:])
        sc.dma_start(cpdump[:], candp[:])
        cv = sb.tile([8, 256], f32)
        for h in range(2):
            nc.sync.dma_start(cv[:, h * 128:(h + 1) * 128],
                              AP(cdump, h * 1024, [[128, 8], [16, 8], [1, 16]]))
            sc.dma_start(AP(atab, h * 256, [[512, 8], [32, 8], [1, 32]]),
                         AP(cpdump, h * 2048, [[256, 8], [32, 8], [1, 32]]))

        # ---- stage 2b: top-16 winning groups per row ---------------------
        wv = sb.tile([8, 16], f32)
        wpos = sb.tile([8, 16], u32)
        cz = sb.tile([8, 256], f32)
        v.max(wv[:, 0:8], cv[:])
        v.max_index(wpos[:, 0:8], wv[:, 0:8], cv[:])
        v.match_replace(cz[:], in_to_replace=wv[:, 0:8], in_values=cv[:],
                        imm_value=NEG)
        v.max(wv[:, 8:16], cz[:])
        v.max_index(wpos[:, 8:16], wv[:, 8:16], cz[:])

        io16 = sb.tile([8, 16], i32)
        gp_.iota(io16[:], pattern=[[0, 16]], base=0, channel_multiplier=512)
        io16f = sb.tile([8, 16], f32)
        v.tensor_copy(io16f[:], io16[:])
        wposf = sb.tile([8, 16], f32)
        v.tensor_copy(wposf[:], wpos[:])
        aofs_f = sb.tile([8, 16], f32)
        v.tensor_scalar(aofs_f[:], wposf[:], 2.0, None, op0=op.mult)
        v.tensor_tensor(aofs_f[:], aofs_f[:], io16f[:], op=op.add)
        aofs = sb.tile([8, 16], i32)
        v.tensor_copy(aofs[:], aofs_f[:])
        nc.sync.dma_start(aod[:], aofs[:])
        aofsq = sb.tile([128, 1], i32)
        nc.sync.dma_start(aofsq[:], AP(aod, 0, [[1, 128], [1, 1]]))

        # gather (off, beam) of each winning group -> partition q=(r,slot)
        attr = sb.tile([128, 2], f32)
        gp_.indirect_dma_start(
            out=attr[:],
            out_offset=None,
            in_=AP(atab, 0, [[1, RPC * 256 * 2], [1, 1]]),
            in_offset=bass.IndirectOffsetOnAxis(ap=aofsq[:, 0:1], axis=0),
        )

        # per-winning-group s and keep via beam-id one-hot
        iom = sb.tile([128, 8], i32)
        gp_.iota(iom[:], pattern=[[1, 8]], base=0, channel_multiplier=0)
        iomf = sb.tile([128, 8], f32)
        v.tensor_copy(iomf[:], iom[:])
        eq = sb.tile([128, 8], f32)
        v.tensor_tensor(eq[:], attr[:, 1:2].to_broadcast([128, 8]), iomf[:],
                        op=op.is_equal)
        tmp8 = sb.tile([128, 8], f32)
        v.tensor_tensor(tmp8[:], eq[:], srow[:], op=op.mult)
        ws = sb.tile([128, 1], f32)
        v.tensor_reduce(ws[:], tmp8[:], axis=X, op=op.add)
        v.tensor_tensor(tmp8[:], eq[:], krow[:], op=op.mult)
        wk = sb.tile([128, 1], f32)
        v.tensor_reduce(wk[:], tmp8[:], axis=X, op=op.add)

        # ---- stage 3: gather winning groups from HBM ---------------------
        gofs_f = sb.tile([128, 1], f32)
        v.tensor_tensor(gofs_f[:], attr[:, 0:1], qc_t[:], op=op.add)
        gofs = sb.tile([128, 1], i32)
        v.tensor_copy(gofs[:], gofs_f[:])
        grp = sb.tile([128, G], f32)
        gp_.indirect_dma_start(
            out=grp[:],
            out_offset=None,
            in_=AP(lp, 0, [[1, RPC * ROWPAD], [1, 1]]),
            in_offset=bass.IndirectOffsetOnAxis(ap=gofs[:, 0:1], axis=0),
        )
        base = sb.tile([128, G], f32)
        sc.activation(base[:], grp[:], mybir.ActivationFunctionType.Identity,
                      bias=ws[:, 0:1], scale=wk[:, 0:1])

        # ---- stage 4a: per-group top-16 ----------------------------------
        fval = sb.tile([128, 16], f32)
        finp = sb.tile([128, 32], f32)     # interleaved (beam, vocab) pairs
        gl = sb.tile([128, 16], u32)
        bz = sb.tile([128, G], f32)
        v.max(fval[:, 0:8], base[:])
        v.max_index(gl[:, 0:8], fval[:, 0:8], base[:])
        v.match_replace(bz[:], in_to_replace=fval[:, 0:8], in_values=base[:],
                        imm_value=NEG)
        v.max(fval[:, 8:16], bz[:])
        v.max_index(gl[:, 8:16], fval[:, 8:16], bz[:])

        glf = sb.tile([128, 16], f32)
        v.tensor_copy(glf[:], gl[:])
        t1 = sb.tile([128, 1], f32)
        v.tensor_scalar(t1[:], attr[:, 1:2], float(VB), None, op0=op.mult)
        vb = sb.tile([128, 1], f32)
        v.tensor_tensor(vb[:], attr[:, 0:1], t1[:], op=op.subtract)
        fpv = finp[:].rearrange("p (k c) -> p c k", c=2)
        v.tensor_copy(fpv[:, 0:1, :].squeeze(1),
                      attr[:, 1:2].to_broadcast([128, 16]))
        v.tensor_scalar(fpv[:, 1:2, :].squeeze(1), glf[:], vb[:, 0:1], None,
                        op0=op.add)

        # ---- bounce final candidates (identity layout) -------------------
        nc.sync.dma_start(fdump[:], fval[:])
        sc.dma_start(fpdump[:], finp[:])
        fv = sb.tile([8, 256], f32)
        nc.sync.dma_start(fv[:], AP(fdump, 0, [[256, 8], [1, 256]]))

        # ---- stage 4c: final top-16 per row ------------------------------
        FV = sb.tile([8, 16], f32)
        fpos = sb.tile([8, 16], u32)
        fz = sb.tile([8, 256], f32)
        v.max(FV[:, 0:8], fv[:])
        v.max_index(fpos[:, 0:8], FV[:, 0:8], fv[:])
        v.match_replace(fz[:], in_to_replace=FV[:, 0:8], in_values=fv[:],
                        imm_value=NEG)
        v.max(FV[:, 8:16], fz[:])
        v.max_index(fpos[:, 8:16], FV[:, 8:16], fz[:])

        fposf = sb.tile([8, 16], f32)
        v.tensor_copy(fposf[:], fpos[:])
        fofs_f = sb.tile([8, 16], f32)
        v.tensor_scalar(fofs_f[:], fposf[:], 2.0, None, op0=op.mult)
        v.tensor_tensor(fofs_f[:], fofs_f[:], io16f[:], op=op.add)
        fofs = sb.tile([8, 16], i32)
        v.tensor_copy(fofs[:], fofs_f[:])
        nc.sync.dma_start(fod[:], fofs[:])
        fofsq = sb.tile([128, 1], i32)
        nc.sync.dma_start(fofsq[:], AP(fod, 0, [[1, 128], [1, 1]]))
        fattr = sb.tile([128, 2], f32)
        gp_.indirect_dma_start(
            out=fattr[:],
            out_offset=None,
            in_=AP(fpdump, 0, [[1, 128 * 32], [1, 1]]),
            in_offset=bass.IndirectOffsetOnAxis(ap=fofsq[:, 0:1], axis=0),
        )
        fbi = sb.tile([128, 1], i32)
        v.tensor_copy(fbi[:], fattr[:, 0:1])
        fvi = sb.tile([128, 1], i32)
        v.tensor_copy(fvi[:], fattr[:, 1:2])

        if debug:
            for nm, t in [("d_gp", gpv), ("d_cand", cand), ("d_candp", candp),
                          ("d_cv", cv), ("d_wv", wv), ("d_attr", attr),
                          ("d_krow", krow), ("d_ws", ws), ("d_wk", wk),
                          ("d_grp", grp), ("d_base", base), ("d_fval", fval),
                          ("d_finp", finp), ("d_fv", fv)]:
                dt_ = nc.dram_tensor(nm, list(t[:].shape), f32,
                                     kind="ExternalOutput")
                sc.dma_start(dt_[:], t[:])
            for nm, t in [("d_aofsq", aofsq), ("d_fofsq", fofsq)]:
                dt_ = nc.dram_tensor(nm, list(t[:].shape), i32,
                                     kind="ExternalOutput")
                sc.dma_start(dt_[:], t[:])

        # ---- outputs -----------------------------------------------------
        nc.sync.dma_start(ov[:], FV[:])
        sc.dma_start(AP(ob, 0, [[1, 128], [1, 1]]), fbi[:])
        nc.sync.dma_start(AP(oi, 0, [[1, 128], [1, 1]]), fvi[:])

        ctx.close()

    nc.compile()
    _CACHE[key] = nc
    return nc


def _prep_inputs(lprobs, scores, mask, step):
    """Host-side shard + marshal. Returns in_maps for the 8 cores."""
    lprobs = np.asarray(lprobs, np.float32)
    scores = np.asarray(scores, np.float32)
    mask = np.ascontiguousarray(np.asarray(mask, np.int32))
    step = int(step)

    if step == 0:
        s2d = np.zeros((BSZ, BEAM), np.float32)
        s2d[:, 1:] = NEG
        mask = mask.copy()
        mask[:, 1:, :] = 0           # force beams 1.. masked with s=NEG
    else:
        s2d = np.ascontiguousarray(scores[:, :, step - 1])

    flat = np.full((BSZ, BEAM, VB), NEG, np.float32)
    flat[:, :, :VOCAB] = lprobs
    flat = flat.reshape(BSZ, ROWPAD)

    pcc, qcc, gidx = _consts()
    p = np.arange(128)
    ph, pr, pm = p // 64, (p // 8) % 8, p % 8      # p = h*64 + r*8 + m
    qr = np.arange(128) // 16                       # q = r*16 + slot
    in_maps = []
    for c in range(NCORES):
        rs = slice(c * RPC, (c + 1) * RPC)
        s_sh = s2d[rs]
        in_maps.append({
            "lp": np.ascontiguousarray(flat[rs]),
            "sv": np.ascontiguousarray(s_sh[pr, pm][:, None]),
            "mk": np.ascontiguousarray(mask[rs][pr, pm]),
            "sq": np.ascontiguousarray(s_sh[qr]),
            "pc": pcc,
            "qc": qcc,
            "gi": gidx,
        })
    return in_maps


def kernel(lprobs, scores, mask, step):
    from concourse.bass_utils import run_bass_kernel_spmd

    nc = _build()
    in_maps = _prep_inputs(lprobs, scores, mask, step)
    res = run_bass_kernel_spmd(nc, in_maps, list(range(NCORES))).results

    vals = np.concatenate([r["ov"] for r in res], axis=0)
    vocab = np.concatenate([r["oi"] for r in res], axis=0)
    beams = np.concatenate([r["ob"] for r in res], axis=0)
    return vals, vocab.astype(np.int32), beams.astype(np.int32)


# revision 15
# speedup vs baseline: 1.5890x; 1.0925x over previous
"""Beam-search top-k (k=16) Trainium2 Bass kernel.

Computation (per batch row b):
  keep[b,m]  = all(mask[b,m,:] != 0)
  val[b,m,v] = keep ? lprobs[b,m,v] + s[b,m] : s[b,m]      (s = scores[:,:,step-1])
  top-16 of val flattened over (m, v), ties -> lowest flat index,
  returning (values, vocab_idx, beam_idx).

Strategy (8 NeuronCores, batch-sharded, 8 rows/core):
  - per-core shard lprobs (8, 8, 50257), each beam padded to VB=50688 with
    -1e30 so every DMA granule is 256B-aligned and beam stride is uniform
  - SBUF layout: 128 partitions = (h half, r row, m beam) x 25344 f32
  - load runs on 6 parallel ~2.1MB DMA streams: SP + Act HWDGE dma_starts
    plus 8 dma_gather calls spread over SWDGE queues 0..3 (each DMA queue
    sustains only ~56 GB/s; six queues ~ 340 GB/s)
  - stage 1: grouped max (g=32) on DVE per 2112-col block -> G (128, 792)
  - stage 2: G' = G*keep + s; per-partition top-16 groups (max8/max_index/
    match_replace x2); candidates bounced through DRAM into per-row (8, 256)
    tables; top-16 winning groups per row
  - stage 3: indirect-DMA gather of the winning 32-elem groups straight
    from HBM (partition q = (row, slot)); re-apply keep/score
  - stage 4: per-group top-16 -> 256 candidates/row -> final top-16/row
  First-match semantics of max_index/match_replace reproduce lax.top_k's
  lowest-index tie-breaking (masked beams are constant rows -> huge ties).
"""

import os
import sys

import numpy as np

sys.path.insert(0, "/opt/trn_rl_repo")

VOCAB = 50257
BEAM = 8
BSZ = 64
NGRAM = 4
K = 16
NCORES = 8
RPC = BSZ // NCORES          # rows (batch entries) per core
VB = 50688                   # padded per-beam length (multiple of 2112)
HALFB = VB // 2              # 25344 elements per partition
ROWPAD = BEAM * VB           # 405504 padded flat row length
G = 32                       # group size (elements)
NGH = HALFB // G             # 792 groups per partition
BLK = 2112                   # load-block columns (= 66 groups)
NBLK = HALFB // BLK          # 12 blocks per partition
NEG = -1.0e30

# SWDGE gather calls: (column_base, width==elem_size, queue).  The whole
# load runs on SWDGE (HWDGE descriptor generation is ~0.5us/descriptor and
# caps a queue near 25 GB/s; SWDGE sprays 16 DMA engines per queue).
# Emission order fixes the DMASW lane (idx % 8); the 3 indirect DMAs that
# follow land on lanes 4..6, so those lanes' gathers must use queue 0.
# Pool-DMA emission order fixes DMASW lanes (idx % 8).  Pool DMAs are:
# gi load (lane 0, queue 0), 13 gathers, then 3 indirect DMAs on lanes
# 6, 7, 0 -> those lanes' gathers must use queue 0.  6336 cols per queue.
GCALLS = [
    (0, 2112, 1), (2112, 2112, 2), (4224, 4224, 3), (8448, 1408, 1),
    (9856, 1408, 2), (11264, 2112, 0), (13376, 2112, 0), (15488, 2112, 0),
    (17600, 1408, 1), (19008, 1408, 2), (20416, 2112, 3), (22528, 1408, 1),
    (23936, 1408, 2),
]

_CACHE = {}


def _consts():
    """Shape-derived constant inputs.  partition p = h*64 + r*8 + m."""
    p = np.arange(128)
    h = p // 64
    m = p % 8
    pc = np.zeros((128, 2), np.float32)
    pc[:, 0] = m * VB + h * HALFB       # element offset of the partition's block
    pc[:, 1] = m                         # beam id
    q = np.arange(128)                   # gathered-partition q = r*16 + slot
    qc = np.zeros((128, 1), np.float32)
    qc[:, 0] = (q // 16) * ROWPAD        # row base for the main gather

    # dma_gather row tables: one call per column-piece, idx wrapped
    # (16, 8) then replicated down the 128 partitions.  The row id of
    # partition p for a call with elem E based at column a is
    # ((p%64)*VB + (p//64)*HALFB) // E  (in_ap view starts at offset a).
    gidx = np.zeros((128, 13 * 8), np.int16)
    for j, (a, E, _q) in enumerate(GCALLS):
        rid = ((np.arange(128) % 64) * VB + (np.arange(128) // 64) * HALFB) // E
        blk = rid.reshape(8, 16).T                  # [q%16, q//16]
        gidx[:, j * 8:(j + 1) * 8] = np.tile(blk, (8, 1))
    return pc, qc, gidx


def _build(debug=False):
    """Build + compile the Bass program (cached per process)."""
    key = ("nc", debug)
    if key in _CACHE:
        return _CACHE[key]

    import concourse.bacc as bacc
    import concourse.tile as tile
    from concourse import mybir
    from concourse.ap import AP
    import concourse.bass as bass

    f32 = mybir.dt.float32
    i16 = mybir.dt.int16
    i32 = mybir.dt.int32
    u32 = mybir.dt.uint32
    X = mybir.AxisListType.X
    op = mybir.AluOpType

    nc = bacc.Bacc("TRN2", target_bir_lowering=False, debug=False,
                   num_devices=NCORES, num_swdge_queues=4)

    lp = nc.dram_tensor("lp", [RPC, ROWPAD], f32, kind="ExternalInput")
    sv = nc.dram_tensor("sv", [128, 1], f32, kind="ExternalInput")
    mk = nc.dram_tensor("mk", [128, NGRAM], i32, kind="ExternalInput")
    sq = nc.dram_tensor("sq", [128, BEAM], f32, kind="ExternalInput")
    pc = nc.dram_tensor("pc", [128, 2], f32, kind="ExternalInput")
    qc = nc.dram_tensor("qc", [128, 1], f32, kind="ExternalInput")
    gi = nc.dram_tensor("gi", [128, 104], i16, kind="ExternalInput")
    ov = nc.dram_tensor("ov", [RPC, K], f32, kind="ExternalOutput")
    oi = nc.dram_tensor("oi", [RPC, K], i32, kind="ExternalOutput")
    ob = nc.dram_tensor("ob", [RPC, K], i32, kind="ExternalOutput")

    cdump = nc.dram_tensor("cdump", [128, 16], f32)    # cand values
    cpdump = nc.dram_tensor("cpdump", [128, 32], f32)  # (off,beam) pairs
    atab = nc.dram_tensor("atab", [RPC, 256, 2], f32)  # per-row (off,beam)
    kdump = nc.dram_tensor("kdump", [128, 1], f32)
    aod = nc.dram_tensor("aod", [RPC, 16], i32)        # attr-offset staging
    fod = nc.dram_tensor("fod", [RPC, 16], i32)        # final-offset staging
    fdump = nc.dram_tensor("fdump", [128, 16], f32)    # final cand values
    fpdump = nc.dram_tensor("fpdump", [128, 32], f32)  # (beam,vocab) pairs

    with tile.TileContext(nc) as tc:
        from contextlib import ExitStack

        ctx = ExitStack()
        sb = ctx.enter_context(tc.tile_pool(name="persist", bufs=1))

        v = nc.vector
        sc = nc.scalar
        gp_ = nc.gpsimd

        # ---- small input loads (off the big-load queues' critical path) --
        sv_t = sb.tile([128, 1], f32)
        sc.dma_start(sv_t[:], sv[:])
        mk_t = sb.tile([128, NGRAM], i32)
        sc.dma_start(mk_t[:], mk[:])
        pc_t = sb.tile([128, 2], f32)
        sc.dma_start(pc_t[:], pc[:])
        qc_t = sb.tile([128, 1], f32)
        sc.dma_start(qc_t[:], qc[:])
        gi_t = sb.tile([128, 104], i16)
        gp_.dma_start(gi_t[:], gi[:])   # pool queue 0: fast + sets lane 0
        srow = sb.tile([128, 8], f32)
        sc.dma_start(srow[:], sq[:])

        # keep = all(mask != 0), as 0.0/1.0; bounce for the q-layout table
        mkf = sb.tile([128, NGRAM], f32)
        v.tensor_copy(mkf[:], mk_t[:])
        keep = sb.tile([128, 1], f32)
        v.tensor_reduce(keep[:], mkf[:], axis=X, op=op.min)
        v.tensor_scalar(keep[:], keep[:], 0.5, None, op0=op.is_ge)
        nc.sync.dma_start(kdump[:], keep[:])
        krow = sb.tile([128, 8], f32)
        sc.dma_start(krow[:], AP(kdump, 0, [[8, 8], [0, 16], [1, 8]]))

        # ---- stage 1: 6-stream load + per-block grouped max --------------
        xt = sb.tile([128, HALFB], f32)
        gg = sb.tile([128, NGH], f32)

        def reduce_blocks(b0, nb):
            v.tensor_reduce(
                gg[:, b0 * (BLK // G):(b0 + nb) * (BLK // G)],
                xt[:, b0 * BLK:(b0 + nb) * BLK].rearrange(
                    "p (n g) -> p n g", g=G),
                axis=X,
                op=op.max,
            )

        def reduce_cols(a, w):
            v.tensor_reduce(
                gg[:, a // G:(a + w) // G],
                xt[:, a:a + w].rearrange("p (n g) -> p n g", g=G),
                axis=X,
                op=op.max,
            )

        # SWDGE gather pieces; emission order fixes the DMASW lanes
        total = RPC * ROWPAD
        for j, (a, E, qn) in enumerate(GCALLS):
            gp_.dma_gather(
                out_ap=xt[:, a:a + E].rearrange("p (i e) -> p i e", i=1),
                in_ap=AP(lp, a, [[E, (total - a) // E], [1, E]]),
                idxs_ap=gi_t[:, j * 8:(j + 1) * 8],
                num_idxs=128,
                num_idxs_reg=128,
                elem_size=E,
                queue_num=qn,
            )
            reduce_cols(a, E)

        # ---- stage 2a: G' and per-partition top-16 groups ----------------
        gpv = sb.tile([128, NGH], f32)
        v.tensor_scalar(gpv[:], gg[:], keep[:, 0:1], sv_t[:, 0:1],
                        op0=op.mult, op1=op.add)
        cand = sb.tile([128, 16], f32)
        candp = sb.tile([128, 32], f32)    # interleaved (off, beam) pairs
        ci = sb.tile([128, 16], u32)
        gz = sb.tile([128, NGH], f32)
        v.max(cand[:, 0:8], gpv[:])
        v.max_index(ci[:, 0:8], cand[:, 0:8], gpv[:])
        v.match_replace(gz[:], in_to_replace=cand[:, 0:8], in_values=gpv[:],
                        imm_value=NEG)
        v.max(cand[:, 8:16], gz[:])
        v.max_index(ci[:, 8:16], cand[:, 8:16], gz[:])

        cif = sb.tile([128, 16], f32)
        v.tensor_copy(cif[:], ci[:])
        cpv = candp[:].rearrange("p (k c) -> p c k", c=2)
        # off = local*32 + (m*VB + h*HALFB)
        v.tensor_scalar(cpv[:, 0:1, :].squeeze(1), cif[:], float(G),
                        pc_t[:, 0:1], op0=op.mult, op1=op.add)
        v.tensor_copy(cpv[:, 1:2, :].squeeze(1),
                      pc_t[:, 1:2].to_broadcast([128, 16]))

        # ---- bounce candidates to per-row layout -------------------------
        nc.sync.dma_start(cdump[:], cand[:])
        sc.dma_start(cpdump[:], candp[:])
        cv = sb.tile([8, 256], f32)
        for h in range(2):
            nc.sync.dma_start(cv[:, h * 128:(h + 1) * 128],
                              AP(cdump, h * 1024, [[128, 8], [16, 8], [1, 16]]))
            sc.dma_start(AP(atab, h * 256, [[512, 8], [32, 8], [1, 32]]),
                         AP(cpdump, h * 2048, [[256, 8], [32, 8], [1, 32]]))

        # ---- stage 2b: top-16 winning groups per row ---------------------
        wv = sb.tile([8, 16], f32)
        wpos = sb.tile([8, 16], u32)
        cz = sb.tile([8, 256], f32)
        v.max(wv[:, 0:8], cv[:])
        v.max_index(wpos[:, 0:8], wv[:, 0:8], cv[:])
        v.match_replace(cz[:], in_to_replace=wv[:, 0:8], in_values=cv[:],
                        imm_value=NEG)
        v.max(wv[:, 8:16], cz[:])
        v.max_index(wpos[:, 8:16], wv[:, 8:16], cz[:])

        io16 = sb.tile([8, 16], i32)
        gp_.iota(io16[:], pattern=[[0, 16]], base=0, channel_multiplier=512)
        io16f = sb.tile([8, 16], f32)
        v.tensor_copy(io16f[:], io16[:])
        wposf = sb.tile([8, 16], f32)
        v.tensor_copy(wposf[:], wpos[:])
        aofs_f = sb.tile([8, 16], f32)
        v.tensor_scalar(aofs_f[:], wposf[:], 2.0, None, op0=op.mult)
        v.tensor_tensor(aofs_f[:], aofs_f[:], io16f[:], op=op.add)
        aofs = sb.tile([8, 16], i32)
        v.tensor_copy(aofs[:], aofs_f[:])
        aofsq = sb.tile([128, 1], i32)
        nc.sync.dma_start(aofsq[:], aofs[:])

        # gather (off, beam) of each winning group -> partition q=(r,slot)
        attr = sb.tile([128, 2], f32)
        gp_.indirect_dma_start(
            out=attr[:],
            out_offset=None,
            in_=AP(atab, 0, [[1, RPC * 256 * 2], [1, 1]]),
            in_offset=bass.IndirectOffsetOnAxis(ap=aofsq[:, 0:1], axis=0),
        )

        # per-winning-group s and keep via beam-id one-hot
        iom = sb.tile([128, 8], i32)
        gp_.iota(iom[:], pattern=[[1, 8]], base=0, channel_multiplier=0)
        iomf = sb.tile([128, 8], f32)
        v.tensor_copy(iomf[:], iom[:])
        eq = sb.tile([128, 8], f32)
        v.tensor_tensor(eq[:], attr[:, 1:2].to_broadcast([128, 8]), iomf[:],
                        op=op.is_equal)
        tmp8 = sb.tile([128, 8], f32)
        v.tensor_tensor(tmp8[:], eq[:], srow[:], op=op.mult)
        ws = sb.tile([128, 1], f32)
        v.tensor_reduce(ws[:], tmp8[:], axis=X, op=op.add)
        v.tensor_tensor(tmp8[:], eq[:], krow[:], op=op.mult)
        wk = sb.tile([128, 1], f32)
        v.tensor_reduce(wk[:], tmp8[:], axis=X, op=op.add)

        # ---- stage 3: gather winning groups from HBM ---------------------
        gofs_f = sb.tile([128, 1], f32)
        v.tensor_tensor(gofs_f[:], attr[:, 0:1], qc_t[:], op=op.add)
        gofs = sb.tile([128, 1], i32)
        v.tensor_copy(gofs[:], gofs_f[:])
        grp = sb.tile([128, G], f32)
        gp_.indirect_dma_start(
            out=grp[:],
            out_offset=None,
            in_=AP(lp, 0, [[1, RPC * ROWPAD], [1, 1]]),
            in_offset=bass.IndirectOffsetOnAxis(ap=gofs[:, 0:1], axis=0),
        )
        base = sb.tile([128, G], f32)
        v.tensor_scalar(base[:], grp[:], wk[:, 0:1], ws[:, 0:1],
                        op0=op.mult, op1=op.add)

        # ---- stage 4a: per-group top-16 ----------------------------------
        fval = sb.tile([128, 16], f32)
        finp = sb.tile([128, 32], f32)     # interleaved (beam, vocab) pairs
        gl = sb.tile([128, 16], u32)
        bz = sb.tile([128, G], f32)
        v.max(fval[:, 0:8], base[:])
        v.max_index(gl[:, 0:8], fval[:, 0:8], base[:])
        v.match_replace(bz[:], in_to_replace=fval[:, 0:8], in_values=base[:],
                        imm_value=NEG)
        v.max(fval[:, 8:16], bz[:])
        v.max_index(gl[:, 8:16], fval[:, 8:16], bz[:])

        glf = sb.tile([128, 16], f32)
        v.tensor_copy(glf[:], gl[:])
        t1 = sb.tile([128, 1], f32)
        v.tensor_scalar(t1[:], attr[:, 1:2], float(VB), None, op0=op.mult)
        vb = sb.tile([128, 1], f32)
        v.tensor_tensor(vb[:], attr[:, 0:1], t1[:], op=op.subtract)
        fpv = finp[:].rearrange("p (k c) -> p c k", c=2)
        v.tensor_copy(fpv[:, 0:1, :].squeeze(1),
                      attr[:, 1:2].to_broadcast([128, 16]))
        v.tensor_scalar(fpv[:, 1:2, :].squeeze(1), glf[:], vb[:, 0:1], None,
                        op0=op.add)

        # ---- bounce final candidates (identity layout) -------------------
        nc.sync.dma_start(fdump[:], fval[:])
        sc.dma_start(fpdump[:], finp[:])
        fv = sb.tile([8, 256], f32)
        nc.sync.dma_start(fv[:], AP(fdump, 0, [[256, 8], [1, 256]]))

        # ---- stage 4c: final top-16 per row ------------------------------
        FV = sb.tile([8, 16], f32)
        fpos = sb.tile([8, 16], u32)
        fz = sb.tile([8, 256], f32)
        v.max(FV[:, 0:8], fv[:])
        v.max_index(fpos[:, 0:8], FV[:, 0:8], fv[:])
        v.match_replace(fz[:], in_to_replace=FV[:, 0:8], in_values=fv[:],
                        imm_value=NEG)
        v.max(FV[:, 8:16], fz[:])
        v.max_index(fpos[:, 8:16], FV[:, 8:16], fz[:])

        fposf = sb.tile([8, 16], f32)
        v.tensor_copy(fposf[:], fpos[:])
        fofs_f = sb.tile([8, 16], f32)
        v.tensor_scalar(fofs_f[:], fposf[:], 2.0, None, op0=op.mult)
        v.tensor_tensor(fofs_f[:], fofs_f[:], io16f[:], op=op.add)
        fofs = sb.tile([8, 16], i32)
        v.tensor_copy(fofs[:], fofs_f[:])
        fofsq = sb.tile([128, 1], i32)
        nc.sync.dma_start(fofsq[:], fofs[:])
        fattr = sb.tile([128, 2], f32)
        gp_.indirect_dma_start(
            out=fattr[:],
            out_offset=None,
            in_=AP(fpdump, 0, [[1, 128 * 32], [1, 1]]),
            in_offset=bass.IndirectOffsetOnAxis(ap=fofsq[:, 0:1], axis=0),
        )
        fbi = sb.tile([128, 1], i32)
        v.tensor_copy(fbi[:], fattr[:, 0:1])
        fvi = sb.tile([128, 1], i32)
        v.tensor_copy(fvi[:], fattr[:, 1:2])

        if debug:
            for nm, t in [("d_gp", gpv), ("d_cand", cand), ("d_candp", candp),
                          ("d_cv", cv), ("d_wv", wv), ("d_attr", attr),
                          ("d_krow", krow), ("d_ws", ws), ("d_wk", wk),
                          ("d_grp", grp), ("d_base", base), ("d_fval", fval),
                          ("d_finp", finp), ("d_fv", fv)]:
                dt_ = nc.dram_tensor(nm, list(t[:].shape), f32,
                                     kind="ExternalOutput")
                sc.dma_start(dt_[:], t[:])
            for nm, t in [("d_aofsq", aofsq), ("d_fofsq", fofsq)]:
                dt_ = nc.dram_tensor(nm, list(t[:].shape), i32,
                                     kind="ExternalOutput")
                sc.dma_start(dt_[:], t[:])

        # ---- outputs -----------------------------------------------------
        nc.sync.dma_start(ov[:], FV[:])
        sc.dma_start(AP(ob, 0, [[1, 128], [1, 1]]), fbi[:])
        nc.sync.dma_start(AP(oi, 0, [[1, 128], [1, 1]]), fvi[:])

        ctx.close()

    nc.compile()
    _CACHE[key] = nc
    return nc


def _prep_inputs(lprobs, scores, mask, step):
    """Host-side shard + marshal. Returns in_maps for the 8 cores."""
    lprobs = np.asarray(lprobs, np.float32)
    scores = np.asarray(scores, np.float32)
    mask = np.ascontiguousarray(np.asarray(mask, np.int32))
    step = int(step)

    if step == 0:
        s2d = np.zeros((BSZ, BEAM), np.float32)
        s2d[:, 1:] = NEG
        mask = mask.copy()
        mask[:, 1:, :] = 0           # force beams 1.. masked with s=NEG
    else:
        s2d = np.ascontiguousarray(scores[:, :, step - 1])

    flat = np.full((BSZ, BEAM, VB), NEG, np.float32)
    flat[:, :, :VOCAB] = lprobs
    flat = flat.reshape(BSZ, ROWPAD)

    pcc, qcc, gidx = _consts()
    p = np.arange(128)
    ph, pr, pm = p // 64, (p // 8) % 8, p % 8      # p = h*64 + r*8 + m
    qr = np.arange(128) // 16                       # q = r*16 + slot
    in_maps = []
    for c in range(NCORES):
        rs = slice(c * RPC, (c + 1) * RPC)
        s_sh = s2d[rs]
        in_maps.append({
            "lp": np.ascontiguousarray(flat[rs]),
            "sv": np.ascontiguousarray(s_sh[pr, pm][:, None]),
            "mk": np.ascontiguousarray(mask[rs][pr, pm]),
            "sq": np.ascontiguousarray(s_sh[qr]),
            "pc": pcc,
            "qc": qcc,
            "gi": gidx,
        })
    return in_maps


def kernel(lprobs, scores, mask, step):
    from concourse.bass_utils import run_bass_kernel_spmd

    nc = _build()
    in_maps = _prep_inputs(lprobs, scores, mask, step)
    res = run_bass_kernel_spmd(nc, in_maps, list(range(NCORES))).results

    vals = np.concatenate([r["ov"] for r in res], axis=0)
    vocab = np.concatenate([r["oi"] for r in res], axis=0)
    beams = np.concatenate([r["ob"] for r in res], axis=0)
    return vals, vocab.astype(np.int32), beams.astype(np.int32)


# revision 16
# speedup vs baseline: 1.6121x; 1.0146x over previous
"""Beam-search top-k (k=16) Trainium2 Bass kernel.

Computation (per batch row b):
  keep[b,m]  = all(mask[b,m,:] != 0)
  val[b,m,v] = keep ? lprobs[b,m,v] + s[b,m] : s[b,m]      (s = scores[:,:,step-1])
  top-16 of val flattened over (m, v), ties -> lowest flat index,
  returning (values, vocab_idx, beam_idx).

Strategy (8 NeuronCores, batch-sharded, 8 rows/core):
  - per-core shard lprobs (8, 8, 50257), each beam padded to VB=50688 with
    -1e30 so every DMA granule is 256B-aligned and beam stride is uniform
  - SBUF layout: 128 partitions = (h half, r row, m beam) x 25344 f32
  - load runs on 6 parallel ~2.1MB DMA streams: SP + Act HWDGE dma_starts
    plus 8 dma_gather calls spread over SWDGE queues 0..3 (each DMA queue
    sustains only ~56 GB/s; six queues ~ 340 GB/s)
  - stage 1: grouped max (g=32) on DVE per 2112-col block -> G (128, 792)
  - stage 2: G' = G*keep + s; per-partition top-16 groups (max8/max_index/
    match_replace x2); candidates bounced through DRAM into per-row (8, 256)
    tables; top-16 winning groups per row
  - stage 3: indirect-DMA gather of the winning 32-elem groups straight
    from HBM (partition q = (row, slot)); re-apply keep/score
  - stage 4: per-group top-16 -> 256 candidates/row -> final top-16/row
  First-match semantics of max_index/match_replace reproduce lax.top_k's
  lowest-index tie-breaking (masked beams are constant rows -> huge ties).
"""

import os
import sys

import numpy as np

sys.path.insert(0, "/opt/trn_rl_repo")

VOCAB = 50257
BEAM = 8
BSZ = 64
NGRAM = 4
K = 16
NCORES = 8
RPC = BSZ // NCORES          # rows (batch entries) per core
VB = 50688                   # padded per-beam length (multiple of 2112)
HALFB = VB // 2              # 25344 elements per partition
ROWPAD = BEAM * VB           # 405504 padded flat row length
G = 64                       # group size (elements)
NGH = HALFB // G             # 792 groups per partition
BLK = 2112                   # load-block columns (= 66 groups)
NBLK = HALFB // BLK          # 12 blocks per partition
NEG = -1.0e30

# SWDGE gather calls: (column_base, width==elem_size, queue).  The whole
# load runs on SWDGE (HWDGE descriptor generation is ~0.5us/descriptor and
# caps a queue near 25 GB/s; SWDGE sprays 16 DMA engines per queue).
# Emission order fixes the DMASW lane (idx % 8); the 3 indirect DMAs that
# follow land on lanes 4..6, so those lanes' gathers must use queue 0.
# Pool-DMA emission order fixes DMASW lanes (idx % 8).  Pool DMAs are:
# gi load (lane 0, queue 0), 13 gathers, then 3 indirect DMAs on lanes
# 6, 7, 0 -> those lanes' gathers must use queue 0.  6336 cols per queue.
GCALLS = [
    (0, 2112, 1), (2112, 2112, 2), (4224, 4224, 3), (8448, 1408, 1),
    (9856, 1408, 2), (11264, 2112, 0), (13376, 2112, 0), (15488, 2112, 0),
    (17600, 1408, 1), (19008, 1408, 2), (20416, 2112, 3), (22528, 1408, 1),
    (23936, 1408, 2),
]

_CACHE = {}


def _consts():
    """Shape-derived constant inputs.  partition p = h*64 + r*8 + m."""
    p = np.arange(128)
    h = p // 64
    m = p % 8
    pc = np.zeros((128, 2), np.float32)
    pc[:, 0] = m * VB + h * HALFB       # element offset of the partition's block
    pc[:, 1] = m                         # beam id
    q = np.arange(128)                   # gathered-partition q = r*16 + slot
    qc = np.zeros((128, 1), np.float32)
    qc[:, 0] = (q // 16) * ROWPAD        # row base for the main gather
    qb = np.tile((np.arange(RPC) * 512).astype(np.float32)[:, None], (1, 16))
    mi = np.tile(np.arange(BEAM, dtype=np.float32)[None, :], (128, 1))

    # dma_gather row tables: one call per column-piece, idx wrapped
    # (16, 8) then replicated down the 128 partitions.  The row id of
    # partition p for a call with elem E based at column a is
    # ((p%64)*VB + (p//64)*HALFB) // E  (in_ap view starts at offset a).
    gidx = np.zeros((128, 13 * 8), np.int16)
    for j, (a, E, _q) in enumerate(GCALLS):
        rid = ((np.arange(128) % 64) * VB + (np.arange(128) // 64) * HALFB) // E
        blk = rid.reshape(8, 16).T                  # [q%16, q//16]
        gidx[:, j * 8:(j + 1) * 8] = np.tile(blk, (8, 1))
    return pc, qc, gidx, qb, mi


def _build(debug=False):
    """Build + compile the Bass program (cached per process)."""
    key = ("nc", debug)
    if key in _CACHE:
        return _CACHE[key]

    import concourse.bacc as bacc
    import concourse.tile as tile
    from concourse import mybir
    from concourse.ap import AP
    import concourse.bass as bass

    f32 = mybir.dt.float32
    i16 = mybir.dt.int16
    i32 = mybir.dt.int32
    u32 = mybir.dt.uint32
    X = mybir.AxisListType.X
    op = mybir.AluOpType

    nc = bacc.Bacc("TRN2", target_bir_lowering=False, debug=False,
                   num_devices=NCORES, num_swdge_queues=4)

    lp = nc.dram_tensor("lp", [RPC, ROWPAD], f32, kind="ExternalInput")
    sv = nc.dram_tensor("sv", [128, 1], f32, kind="ExternalInput")
    mk = nc.dram_tensor("mk", [128, NGRAM], i32, kind="ExternalInput")
    sq = nc.dram_tensor("sq", [128, BEAM], f32, kind="ExternalInput")
    pc = nc.dram_tensor("pc", [128, 2], f32, kind="ExternalInput")
    qc = nc.dram_tensor("qc", [128, 1], f32, kind="ExternalInput")
    gi = nc.dram_tensor("gi", [128, 104], i16, kind="ExternalInput")
    qb = nc.dram_tensor("qb", [RPC, 16], f32, kind="ExternalInput")
    mi = nc.dram_tensor("mi", [128, BEAM], f32, kind="ExternalInput")
    ov = nc.dram_tensor("ov", [RPC, K], f32, kind="ExternalOutput")
    oi = nc.dram_tensor("oi", [RPC, K], i32, kind="ExternalOutput")
    ob = nc.dram_tensor("ob", [RPC, K], i32, kind="ExternalOutput")

    crow = nc.dram_tensor("crow", [RPC, 256], f32)     # cand values per row
    atab = nc.dram_tensor("atab", [RPC, 256, 2], f32)  # per-row (off,beam)
    kdump = nc.dram_tensor("kdump", [128, 1], f32)
    aod = nc.dram_tensor("aod", [RPC, 16], i32)        # attr-offset staging
    fod = nc.dram_tensor("fod", [RPC, 16], i32)        # final-offset staging
    fdump = nc.dram_tensor("fdump", [128, 16], f32)    # final cand values
    fpdump = nc.dram_tensor("fpdump", [128, 32], f32)  # (beam,vocab) pairs

    with tile.TileContext(nc) as tc:
        from contextlib import ExitStack

        ctx = ExitStack()
        sb = ctx.enter_context(tc.tile_pool(name="persist", bufs=1))

        v = nc.vector
        sc = nc.scalar
        gp_ = nc.gpsimd

        # ---- small input loads (off the big-load queues' critical path) --
        sv_t = sb.tile([128, 1], f32)
        sc.dma_start(sv_t[:], sv[:])
        mk_t = sb.tile([128, NGRAM], i32)
        sc.dma_start(mk_t[:], mk[:])
        pc_t = sb.tile([128, 2], f32)
        sc.dma_start(pc_t[:], pc[:])
        qc_t = sb.tile([128, 1], f32)
        sc.dma_start(qc_t[:], qc[:])
        gi_t = sb.tile([128, 104], i16)
        gp_.dma_start(gi_t[:], gi[:])   # pool queue 0: fast + sets lane 0
        srow = sb.tile([128, 8], f32)
        sc.dma_start(srow[:], sq[:])
        io16f = sb.tile([8, 16], f32)
        sc.dma_start(io16f[:], qb[:])
        iomf = sb.tile([128, 8], f32)
        sc.dma_start(iomf[:], mi[:])

        # keep = all(mask != 0), as 0.0/1.0; bounce for the q-layout table
        mkf = sb.tile([128, NGRAM], f32)
        v.tensor_copy(mkf[:], mk_t[:])
        keep = sb.tile([128, 1], f32)
        v.tensor_reduce(keep[:], mkf[:], axis=X, op=op.min)
        v.tensor_scalar(keep[:], keep[:], 0.5, None, op0=op.is_ge)

        # ---- stage 1: 6-stream load + per-block grouped max --------------
        xt = sb.tile([128, HALFB], f32)
        gg = sb.tile([128, NGH], f32)

        def reduce_blocks(b0, nb):
            v.tensor_reduce(
                gg[:, b0 * (BLK // G):(b0 + nb) * (BLK // G)],
                xt[:, b0 * BLK:(b0 + nb) * BLK].rearrange(
                    "p (n g) -> p n g", g=G),
                axis=X,
                op=op.max,
            )

        def reduce_cols(a, w):
            v.tensor_reduce(
                gg[:, a // G:(a + w) // G],
                xt[:, a:a + w].rearrange("p (n g) -> p n g", g=G),
                axis=X,
                op=op.max,
            )

        # SWDGE gather pieces; emission order fixes the DMASW lanes
        total = RPC * ROWPAD
        r128 = gp_.to_reg(128)
        for j, (a, E, qn) in enumerate(GCALLS):
            gp_.dma_gather(
                out_ap=xt[:, a:a + E].rearrange("p (i e) -> p i e", i=1),
                in_ap=AP(lp, a, [[E, (total - a) // E], [1, E]]),
                idxs_ap=gi_t[:, j * 8:(j + 1) * 8],
                num_idxs=128,
                num_idxs_reg=r128,
                elem_size=E,
                queue_num=qn,
            )
            reduce_cols(a, E)

        # ---- stage 2a: G' and per-partition top-16 groups ----------------
        gpv = sb.tile([128, NGH], f32)
        v.tensor_scalar(gpv[:], gg[:], keep[:, 0:1], sv_t[:, 0:1],
                        op0=op.mult, op1=op.add)
        cand = sb.tile([128, 16], f32)
        candp = sb.tile([128, 32], f32)    # interleaved (off, beam) pairs
        ci = sb.tile([128, 16], u32)
        gz = sb.tile([128, NGH], f32)
        v.max(cand[:, 0:8], gpv[:])
        v.max_index(ci[:, 0:8], cand[:, 0:8], gpv[:])
        v.match_replace(gz[:], in_to_replace=cand[:, 0:8], in_values=gpv[:],
                        imm_value=NEG)
        v.max(cand[:, 8:16], gz[:])
        v.max_index(ci[:, 8:16], cand[:, 8:16], gz[:])

        cif = sb.tile([128, 16], f32)
        v.tensor_copy(cif[:], ci[:])
        cpv = candp[:].rearrange("p (k c) -> p c k", c=2)
        # off = local*32 + (m*VB + h*HALFB)
        v.tensor_scalar(cpv[:, 0:1, :].squeeze(1), cif[:], float(G),
                        pc_t[:, 0:1], op0=op.mult, op1=op.add)
        v.tensor_copy(cpv[:, 1:2, :].squeeze(1),
                      pc_t[:, 1:2].to_broadcast([128, 16]))

        # ---- bounce candidates to per-row layout (write-side reorder) ----
        for h in range(2):
            nc.sync.dma_start(AP(crow, h * 128, [[256, 8], [16, 8], [1, 16]]),
                              cand[h * 64:(h + 1) * 64, :])
            sc.dma_start(AP(atab, h * 256, [[512, 8], [32, 8], [1, 32]]),
                         candp[h * 64:(h + 1) * 64, :])
        cv = sb.tile([8, 256], f32)
        nc.sync.dma_start(cv[:], AP(crow, 0, [[256, 8], [1, 256]]))

        # ---- stage 2b: top-16 winning groups per row ---------------------
        wv = sb.tile([8, 16], f32)
        wpos = sb.tile([8, 16], u32)
        cz = sb.tile([8, 256], f32)
        v.max(wv[:, 0:8], cv[:])
        v.max_index(wpos[:, 0:8], wv[:, 0:8], cv[:])
        v.match_replace(cz[:], in_to_replace=wv[:, 0:8], in_values=cv[:],
                        imm_value=NEG)
        v.max(wv[:, 8:16], cz[:])
        v.max_index(wpos[:, 8:16], wv[:, 8:16], cz[:])

        wposf = sb.tile([8, 16], f32)
        v.tensor_copy(wposf[:], wpos[:])
        aofs_f = sb.tile([8, 16], f32)
        v.tensor_scalar(aofs_f[:], wposf[:], 2.0, None, op0=op.mult)
        v.tensor_tensor(aofs_f[:], aofs_f[:], io16f[:], op=op.add)
        aofs = sb.tile([8, 16], i32)
        v.tensor_copy(aofs[:], aofs_f[:])
        aofsq = sb.tile([128, 1], i32)
        nc.sync.dma_start(aofsq[:], aofs[:])

        # gather (off, beam) of each winning group -> partition q=(r,slot)
        attr = sb.tile([128, 2], f32)
        gp_.indirect_dma_start(
            out=attr[:],
            out_offset=None,
            in_=AP(atab, 0, [[1, RPC * 256 * 2], [1, 1]]),
            in_offset=bass.IndirectOffsetOnAxis(ap=aofsq[:, 0:1], axis=0),
        )

        # per-winning-group s and keep via beam-id one-hot
        nc.sync.dma_start(kdump[:], keep[:])
        krow = sb.tile([128, 8], f32)
        sc.dma_start(krow[:], AP(kdump, 0, [[8, 8], [0, 16], [1, 8]]))
        eq = sb.tile([128, 8], f32)
        v.tensor_tensor(eq[:], attr[:, 1:2].to_broadcast([128, 8]), iomf[:],
                        op=op.is_equal)
        tmp8 = sb.tile([128, 8], f32)
        v.tensor_tensor(tmp8[:], eq[:], srow[:], op=op.mult)
        ws = sb.tile([128, 1], f32)
        v.tensor_reduce(ws[:], tmp8[:], axis=X, op=op.add)
        v.tensor_tensor(tmp8[:], eq[:], krow[:], op=op.mult)
        wk = sb.tile([128, 1], f32)
        v.tensor_reduce(wk[:], tmp8[:], axis=X, op=op.add)

        # ---- stage 3: gather winning groups from HBM ---------------------
        gofs_f = sb.tile([128, 1], f32)
        v.tensor_tensor(gofs_f[:], attr[:, 0:1], qc_t[:], op=op.add)
        gofs = sb.tile([128, 1], i32)
        v.tensor_copy(gofs[:], gofs_f[:])
        grp = sb.tile([128, G], f32)
        gp_.indirect_dma_start(
            out=grp[:],
            out_offset=None,
            in_=AP(lp, 0, [[1, RPC * ROWPAD], [1, 1]]),
            in_offset=bass.IndirectOffsetOnAxis(ap=gofs[:, 0:1], axis=0),
        )
        base = sb.tile([128, G], f32)
        v.tensor_scalar(base[:], grp[:], wk[:, 0:1], ws[:, 0:1],
                        op0=op.mult, op1=op.add)

        # ---- stage 4a: per-group top-16 ----------------------------------
        fval = sb.tile([128, 16], f32)
        finp = sb.tile([128, 32], f32)     # interleaved (beam, vocab) pairs
        gl = sb.tile([128, 16], u32)
        bz = sb.tile([128, G], f32)
        v.max(fval[:, 0:8], base[:])
        v.max_index(gl[:, 0:8], fval[:, 0:8], base[:])
        v.match_replace(bz[:], in_to_replace=fval[:, 0:8], in_values=base[:],
                        imm_value=NEG)
        v.max(fval[:, 8:16], bz[:])
        v.max_index(gl[:, 8:16], fval[:, 8:16], bz[:])

        glf = sb.tile([128, 16], f32)
        v.tensor_copy(glf[:], gl[:])
        t1 = sb.tile([128, 1], f32)
        v.tensor_scalar(t1[:], attr[:, 1:2], float(VB), None, op0=op.mult)
        vb = sb.tile([128, 1], f32)
        v.tensor_tensor(vb[:], attr[:, 0:1], t1[:], op=op.subtract)
        fpv = finp[:].rearrange("p (k c) -> p c k", c=2)
        v.tensor_copy(fpv[:, 0:1, :].squeeze(1),
                      attr[:, 1:2].to_broadcast([128, 16]))
        v.tensor_scalar(fpv[:, 1:2, :].squeeze(1), glf[:], vb[:, 0:1], None,
                        op0=op.add)

        # ---- bounce final candidates (identity layout) -------------------
        nc.sync.dma_start(fdump[:], fval[:])
        sc.dma_start(fpdump[:], finp[:])
        fv = sb.tile([8, 256], f32)
        nc.sync.dma_start(fv[:], AP(fdump, 0, [[256, 8], [1, 256]]))

        # ---- stage 4c: final top-16 per row ------------------------------
        FV = sb.tile([8, 16], f32)
        fpos = sb.tile([8, 16], u32)
        fz = sb.tile([8, 256], f32)
        v.max(FV[:, 0:8], fv[:])
        v.max_index(fpos[:, 0:8], FV[:, 0:8], fv[:])
        v.match_replace(fz[:], in_to_replace=FV[:, 0:8], in_values=fv[:],
                        imm_value=NEG)
        v.max(FV[:, 8:16], fz[:])
        v.max_index(fpos[:, 8:16], FV[:, 8:16], fz[:])

        fposf = sb.tile([8, 16], f32)
        v.tensor_copy(fposf[:], fpos[:])
        fofs_f = sb.tile([8, 16], f32)
        v.tensor_scalar(fofs_f[:], fposf[:], 2.0, None, op0=op.mult)
        v.tensor_tensor(fofs_f[:], fofs_f[:], io16f[:], op=op.add)
        fofs = sb.tile([8, 16], i32)
        v.tensor_copy(fofs[:], fofs_f[:])
        fofsq = sb.tile([128, 1], i32)
        nc.sync.dma_start(fofsq[:], fofs[:])
        fattr = sb.tile([128, 2], f32)
        gp_.indirect_dma_start(
            out=fattr[:],
            out_offset=None,
            in_=AP(fpdump, 0, [[1, 128 * 32], [1, 1]]),
            in_offset=bass.IndirectOffsetOnAxis(ap=fofsq[:, 0:1], axis=0),
        )
        fbi = sb.tile([128, 1], i32)
        v.tensor_copy(fbi[:], fattr[:, 0:1])
        fvi = sb.tile([128, 1], i32)
        v.tensor_copy(fvi[:], fattr[:, 1:2])

        if debug:
            for nm, t in [("d_gp", gpv), ("d_cand", cand), ("d_candp", candp),
                          ("d_cv", cv), ("d_wv", wv), ("d_attr", attr),
                          ("d_krow", krow), ("d_ws", ws), ("d_wk", wk),
                          ("d_grp", grp), ("d_base", base), ("d_fval", fval),
                          ("d_finp", finp), ("d_fv", fv)]:
                dt_ = nc.dram_tensor(nm, list(t[:].shape), f32,
                                     kind="ExternalOutput")
                sc.dma_start(dt_[:], t[:])
            for nm, t in [("d_aofsq", aofsq), ("d_fofsq", fofsq)]:
                dt_ = nc.dram_tensor(nm, list(t[:].shape), i32,
                                     kind="ExternalOutput")
                sc.dma_start(dt_[:], t[:])

        # ---- outputs -----------------------------------------------------
        nc.sync.dma_start(ov[:], FV[:])
        sc.dma_start(AP(ob, 0, [[1, 128], [1, 1]]), fbi[:])
        nc.sync.dma_start(AP(oi, 0, [[1, 128], [1, 1]]), fvi[:])

        ctx.close()

    nc.compile()
    _CACHE[key] = nc
    return nc


def _prep_inputs(lprobs, scores, mask, step):
    """Host-side shard + marshal. Returns in_maps for the 8 cores."""
    lprobs = np.asarray(lprobs, np.float32)
    scores = np.asarray(scores, np.float32)
    mask = np.ascontiguousarray(np.asarray(mask, np.int32))
    step = int(step)

    if step == 0:
        s2d = np.zeros((BSZ, BEAM), np.float32)
        s2d[:, 1:] = NEG
        mask = mask.copy()
        mask[:, 1:, :] = 0           # force beams 1.. masked with s=NEG
    else:
        s2d = np.ascontiguousarray(scores[:, :, step - 1])

    flat = np.full((BSZ, BEAM, VB), NEG, np.float32)
    flat[:, :, :VOCAB] = lprobs
    flat = flat.reshape(BSZ, ROWPAD)

    pcc, qcc, gidx, qbb, mii = _consts()
    p = np.arange(128)
    ph, pr, pm = p // 64, (p // 8) % 8, p % 8      # p = h*64 + r*8 + m
    qr = np.arange(128) // 16                       # q = r*16 + slot
    in_maps = []
    for c in range(NCORES):
        rs = slice(c * RPC, (c + 1) * RPC)
        s_sh = s2d[rs]
        in_maps.append({
            "lp": np.ascontiguousarray(flat[rs]),
            "sv": np.ascontiguousarray(s_sh[pr, pm][:, None]),
            "mk": np.ascontiguousarray(mask[rs][pr, pm]),
            "sq": np.ascontiguousarray(s_sh[qr]),
            "pc": pcc,
            "qc": qcc,
            "gi": gidx,
            "qb": qbb,
            "mi": mii,
        })
    return in_maps


def kernel(lprobs, scores, mask, step):
    from concourse.bass_utils import run_bass_kernel_spmd

    nc = _build()
    in_maps = _prep_inputs(lprobs, scores, mask, step)
    res = run_bass_kernel_spmd(nc, in_maps, list(range(NCORES))).results

    vals = np.concatenate([r["ov"] for r in res], axis=0)
    vocab = np.concatenate([r["oi"] for r in res], axis=0)
    beams = np.concatenate([r["ob"] for r in res], axis=0)
    return vals, vocab.astype(np.int32), beams.astype(np.int32)
